# revision 3
# baseline (speedup 1.0000x reference)
"""DynamicGraphEmbedding kernel for 8 Trainium2 NeuronCores.

The reference collapses algebraically:
  - deg[i] == K == 16 for every node (dst list is repeat(arange(N), K)),
    so gcn_norm edge weight ew == 1/16 for every edge.
  - straight-through gumbel gate is exactly y_hard in the forward pass,
    i.e. gate(e) = 1 iff argmax(softmax(logits[e] + g[e])) == 0.
  - therefore out[b] = A @ (x[b] @ W) + bias, with the dense [N, N] matrix
    A[i, j] = gate(i*N+j)/16 if j in topk_j[i] else 0.

Host (tiny, O(N^2)): build A from emb/logits/gumbel_u with the exact same
jax-on-CPU ops as the reference. Device (the memory-bound bulk): two chained
256^3 matmuls per batch element, data-parallel over batch across 8 cores.
"""

import sys

import numpy as np

if "/opt/trn_rl_repo" not in sys.path:
    sys.path.insert(0, "/opt/trn_rl_repo")

N, T, B, D, K = 256, 256, 64, 64, 16
NCORES = 8
BPC = B // NCORES  # batch elements per core

_CACHE = {}
LAST_RESULT = None  # BassKernelResults of the most recent run (for profiling)


def _graph_matrix(emb, logits, gumbel_u):
    """Dense [N, N] combined gate/topk/gcn-norm matrix A (host-side, tiny)."""
    try:
        import jax
        import jax.numpy as jnp

        cpu = jax.devices("cpu")[0]
        emb_j = jax.device_put(np.asarray(emb), cpu)
        logits_j = jax.device_put(np.asarray(logits), cpu)
        gu_j = jax.device_put(np.asarray(gumbel_u), cpu)
        nrm = jnp.linalg.norm(emb_j, axis=-1)
        cos = (emb_j @ emb_j.T) / (nrm[:, None] * nrm[None, :])
        _, topk_j = jax.lax.top_k(cos, K)
        g = -jnp.log(-jnp.log(gu_j))
        y_soft = jax.nn.softmax(logits_j + g, axis=-1)
        am = jnp.argmax(y_soft, axis=-1)
        topk = np.asarray(topk_j)
        gate_full = (np.asarray(am) == 0).astype(np.float32)
    except Exception:
        emb32 = np.asarray(emb, np.float32)
        nrm = np.sqrt((emb32 * emb32).sum(-1))
        cos = (emb32 @ emb32.T) / (nrm[:, None] * nrm[None, :])
        topk = np.argsort(-cos, axis=-1, kind="stable")[:, :K]
        lg = np.asarray(logits, np.float32) + np.float32(-1.0) * np.log(
            -np.log(np.asarray(gumbel_u, np.float32))
        )
        e = np.exp(lg - lg.max(-1, keepdims=True))
        y_soft = e / e.sum(-1, keepdims=True)
        gate_full = (np.argmax(y_soft, -1) == 0).astype(np.float32)
    rows = np.repeat(np.arange(N), K)
    cols = topk.reshape(-1)
    A = np.zeros((N, N), np.float32)
    A[rows, cols] = gate_full[rows * N + cols] * np.float32(0.0625)
    return A


def _build_bass():
    """Per-core Bass graph: out[b] = A @ (x[b] @ W) + bias for BPC batches.

    Inputs are laid out so no on-device transposes are needed:
      xT  [BPC, T, N]  x shard, transposed per batch on host
      AT  [N, N]       A transposed (contraction index on rows)
      W   [T, T]       natural
      bias [1, T]
      out [BPC, N, T]
    """
    import concourse.bass as bass
    import concourse.mybir as mybir
    from concourse import bacc
    from concourse.tile import TileContext

    F32 = mybir.dt.float32

    nc = bacc.Bacc()
    xT = nc.declare_dram_parameter("xT", [BPC, T, N], F32, isOutput=False)
    AT = nc.declare_dram_parameter("AT", [N, N], F32, isOutput=False)
    Wp = nc.declare_dram_parameter("W", [T, T], F32, isOutput=False)
    bp = nc.declare_dram_parameter("bias", [1, T], F32, isOutput=False)
    out = nc.declare_dram_parameter("out", [BPC, N, T], F32, isOutput=True)

    with TileContext(nc) as tc:
        with (
            tc.tile_pool(name="const", bufs=1) as const,
            tc.tile_pool(name="xin", bufs=3) as xin,
            tc.tile_pool(name="hbuf", bufs=2) as hbuf,
            tc.tile_pool(name="obuf", bufs=4) as obuf,
            tc.tile_pool(name="psum", bufs=2, space="PSUM") as psum,
        ):
            # Constants: W and AT as [128, chunk, 256] (partition = contraction
            # index within chunk), bias broadcast to all 128 partitions.
            w_sb = const.tile([128, 2, T], F32)
            nc.sync.dma_start(out=w_sb, in_=Wp.rearrange("(c p) t -> p c t", p=128))
            a_sb = const.tile([128, 2, N], F32)
            nc.sync.dma_start(out=a_sb, in_=AT.rearrange("(c p) n -> p c n", p=128))
            bias_bc = const.tile([128, T], F32)
            nc.gpsimd.dma_start(out=bias_bc, in_=bp.ap().to_broadcast([128, T]))

            for b in range(BPC):
                # x[b]^T as [t_in_chunk, chunk, n]
                xt = xin.tile([128, 2, N], F32)
                nc.sync.dma_start(
                    out=xt, in_=xT[b].rearrange("(c p) n -> p c n", p=128)
                )
                # h[b] = x[b] @ W, row blocks m of 128 nodes
                h_sb = hbuf.tile([128, 2, T], F32)  # [j_in_chunk, j_chunk, t']
                for m in range(2):
                    ph = psum.tile([128, T], F32)
                    nc.tensor.matmul(
                        ph,
                        lhsT=xt[:, 0, bass.ts(m, 128)],
                        rhs=w_sb[:, 0, :],
                        start=True,
                        stop=False,
                    )
                    nc.tensor.matmul(
                        ph,
                        lhsT=xt[:, 1, bass.ts(m, 128)],
                        rhs=w_sb[:, 1, :],
                        start=False,
                        stop=True,
                    )
                    nc.vector.tensor_copy(h_sb[:, m, :], ph)
                # out[b] rows block m: sum over j chunks of AT
                for m in range(2):
                    po = psum.tile([128, T], F32)
                    nc.tensor.matmul(
                        po,
                        lhsT=a_sb[:, 0, bass.ts(m, 128)],
                        rhs=h_sb[:, 0, :],
                        start=True,
                        stop=False,
                    )
                    nc.tensor.matmul(
                        po,
                        lhsT=a_sb[:, 1, bass.ts(m, 128)],
                        rhs=h_sb[:, 1, :],
                        start=False,
                        stop=True,
                    )
                    o_sb = obuf.tile([128, T], F32)
                    nc.vector.tensor_add(o_sb, po, bias_bc)
                    nc.sync.dma_start(out=out[b, bass.ts(m, 128), :], in_=o_sb)
    nc.finalize()
    return nc


def kernel(x, emb, W, b, logits, gumbel_u):
    global LAST_RESULT
    from concourse.bass_utils import run_bass_kernel_spmd

    x = np.ascontiguousarray(np.asarray(x, np.float32))
    W = np.ascontiguousarray(np.asarray(W, np.float32))
    bias = np.ascontiguousarray(np.asarray(b, np.float32)).reshape(1, T)

    A = _graph_matrix(emb, logits, gumbel_u)
    AT = np.ascontiguousarray(A.T)
    xT = np.ascontiguousarray(x.transpose(0, 2, 1))  # [B, T, N]

    if "nc" not in _CACHE:
        _CACHE["nc"] = _build_bass()
    nc = _CACHE["nc"]

    in_maps = [
        {"xT": xT[c * BPC : (c + 1) * BPC], "AT": AT, "W": W, "bias": bias}
        for c in range(NCORES)
    ]
    res = run_bass_kernel_spmd(nc, in_maps, core_ids=list(range(NCORES)))
    LAST_RESULT = res
    out = np.concatenate([res.results[c]["out"] for c in range(NCORES)], axis=0)
    return out


# revision 4
# speedup vs baseline: 1.3237x; 1.3237x over previous
"""DynamicGraphEmbedding kernel for 8 Trainium2 NeuronCores.

The reference collapses algebraically:
  - deg[i] == K == 16 for every node (dst list is repeat(arange(N), K)),
    so gcn_norm edge weight ew == 1/16 for every edge.
  - straight-through gumbel gate is exactly y_hard in the forward pass,
    i.e. gate(e) = 1 iff argmax(softmax(logits[e] + g[e])) == 0.
  - therefore out[b] = A @ (x[b] @ W) + bias, with the dense [N, N] matrix
    A[i, j] = gate(i*N+j)/16 if j in topk_j[i] else 0.

Host (tiny, O(N^2)): build A from emb/logits/gumbel_u with the exact same
jax-on-CPU ops as the reference. Device (the memory-bound bulk): two chained
256^3 matmuls per batch element, data-parallel over batch across 8 cores.
"""

import sys

import numpy as np

if "/opt/trn_rl_repo" not in sys.path:
    sys.path.insert(0, "/opt/trn_rl_repo")

N, T, B, D, K = 256, 256, 64, 64, 16
NCORES = 8
BPC = B // NCORES  # batch elements per core

_CACHE = {}
LAST_RESULT = None  # BassKernelResults of the most recent run (for profiling)


def _graph_matrix(emb, logits, gumbel_u):
    """Dense [N, N] combined gate/topk/gcn-norm matrix A (host-side, tiny)."""
    try:
        import jax
        import jax.numpy as jnp

        cpu = jax.devices("cpu")[0]
        emb_j = jax.device_put(np.asarray(emb), cpu)
        logits_j = jax.device_put(np.asarray(logits), cpu)
        gu_j = jax.device_put(np.asarray(gumbel_u), cpu)
        nrm = jnp.linalg.norm(emb_j, axis=-1)
        cos = (emb_j @ emb_j.T) / (nrm[:, None] * nrm[None, :])
        _, topk_j = jax.lax.top_k(cos, K)
        g = -jnp.log(-jnp.log(gu_j))
        y_soft = jax.nn.softmax(logits_j + g, axis=-1)
        am = jnp.argmax(y_soft, axis=-1)
        topk = np.asarray(topk_j)
        gate_full = (np.asarray(am) == 0).astype(np.float32)
    except Exception:
        emb32 = np.asarray(emb, np.float32)
        nrm = np.sqrt((emb32 * emb32).sum(-1))
        cos = (emb32 @ emb32.T) / (nrm[:, None] * nrm[None, :])
        topk = np.argsort(-cos, axis=-1, kind="stable")[:, :K]
        lg = np.asarray(logits, np.float32) + np.float32(-1.0) * np.log(
            -np.log(np.asarray(gumbel_u, np.float32))
        )
        e = np.exp(lg - lg.max(-1, keepdims=True))
        y_soft = e / e.sum(-1, keepdims=True)
        gate_full = (np.argmax(y_soft, -1) == 0).astype(np.float32)
    rows = np.repeat(np.arange(N), K)
    cols = topk.reshape(-1)
    A = np.zeros((N, N), np.float32)
    A[rows, cols] = gate_full[rows * N + cols] * np.float32(0.0625)
    return A


def _build_bass():
    """Per-core Bass graph: out[b] = A @ (x[b] @ W) + bias for BPC batches.

    Inputs are laid out so no on-device transposes are needed:
      xT  [BPC, T, N]  x shard, transposed per batch on host
      AT  [N, N]       A transposed (contraction index on rows)
      W   [T, T]       natural
      bias [1, T]
      out [BPC, N, T]
    """
    import concourse.bass as bass
    import concourse.mybir as mybir
    from concourse import bacc
    from concourse.tile import TileContext

    F32 = mybir.dt.float32
    # float32r: single-pass PE fp32 (TF32-ish rounding, ~1e-4 rel err) at 4x
    # the throughput of the 2-pass float32 path. PSUM accumulation stays f32.
    MMDT = mybir.dt.float32r

    nc = bacc.Bacc()
    xT = nc.declare_dram_parameter("xT", [BPC, T, N], MMDT, isOutput=False)
    AT = nc.declare_dram_parameter("AT", [N, N], MMDT, isOutput=False)
    Wp = nc.declare_dram_parameter("W", [T, T], MMDT, isOutput=False)
    bp = nc.declare_dram_parameter("bias", [1, T], F32, isOutput=False)
    out = nc.declare_dram_parameter("out", [BPC, N, T], F32, isOutput=True)

    with TileContext(nc) as tc:
        with (
            tc.tile_pool(name="const", bufs=1) as const,
            tc.tile_pool(name="xin", bufs=3) as xin,
            tc.tile_pool(name="hbuf", bufs=2) as hbuf,
            tc.tile_pool(name="obuf", bufs=4) as obuf,
            tc.tile_pool(name="psum", bufs=2, space="PSUM") as psum,
        ):
            # Constants: W and AT as [128, chunk, 256] (partition = contraction
            # index within chunk), bias broadcast to all 128 partitions.
            w_sb = const.tile([128, 2, T], MMDT)
            nc.sync.dma_start(out=w_sb, in_=Wp.rearrange("(c p) t -> p c t", p=128))
            a_sb = const.tile([128, 2, N], MMDT)
            nc.sync.dma_start(out=a_sb, in_=AT.rearrange("(c p) n -> p c n", p=128))
            bias_bc = const.tile([128, T], F32)
            nc.gpsimd.dma_start(out=bias_bc, in_=bp.ap().to_broadcast([128, T]))

            for b in range(BPC):
                # x[b]^T as [t_in_chunk, chunk, n]
                xt = xin.tile([128, 2, N], MMDT)
                nc.sync.dma_start(
                    out=xt, in_=xT[b].rearrange("(c p) n -> p c n", p=128)
                )
                # h[b] = x[b] @ W, row blocks m of 128 nodes
                h_sb = hbuf.tile([128, 2, T], MMDT)  # [j_in_chunk, j_chunk, t']
                for m in range(2):
                    ph = psum.tile([128, T], F32)
                    nc.tensor.matmul(
                        ph,
                        lhsT=xt[:, 0, bass.ts(m, 128)],
                        rhs=w_sb[:, 0, :],
                        start=True,
                        stop=False,
                    )
                    nc.tensor.matmul(
                        ph,
                        lhsT=xt[:, 1, bass.ts(m, 128)],
                        rhs=w_sb[:, 1, :],
                        start=False,
                        stop=True,
                    )
                    nc.vector.tensor_copy(h_sb[:, m, :], ph)
                # out[b] rows block m: sum over j chunks of AT
                for m in range(2):
                    po = psum.tile([128, T], F32)
                    nc.tensor.matmul(
                        po,
                        lhsT=a_sb[:, 0, bass.ts(m, 128)],
                        rhs=h_sb[:, 0, :],
                        start=True,
                        stop=False,
                    )
                    nc.tensor.matmul(
                        po,
                        lhsT=a_sb[:, 1, bass.ts(m, 128)],
                        rhs=h_sb[:, 1, :],
                        start=False,
                        stop=True,
                    )
                    o_sb = obuf.tile([128, T], F32)
                    nc.vector.tensor_add(o_sb, po, bias_bc)
                    nc.sync.dma_start(out=out[b, bass.ts(m, 128), :], in_=o_sb)
    nc.finalize()
    return nc


def kernel(x, emb, W, b, logits, gumbel_u):
    global LAST_RESULT
    from concourse.bass_utils import run_bass_kernel_spmd

    x = np.ascontiguousarray(np.asarray(x, np.float32))
    W = np.ascontiguousarray(np.asarray(W, np.float32))
    bias = np.ascontiguousarray(np.asarray(b, np.float32)).reshape(1, T)

    A = _graph_matrix(emb, logits, gumbel_u)
    AT = np.ascontiguousarray(A.T)
    xT = np.ascontiguousarray(x.transpose(0, 2, 1))  # [B, T, N]

    if "nc" not in _CACHE:
        _CACHE["nc"] = _build_bass()
    nc = _CACHE["nc"]

    in_maps = [
        {"xT": xT[c * BPC : (c + 1) * BPC], "AT": AT, "W": W, "bias": bias}
        for c in range(NCORES)
    ]
    res = run_bass_kernel_spmd(nc, in_maps, core_ids=list(range(NCORES)))
    LAST_RESULT = res
    out = np.concatenate([res.results[c]["out"] for c in range(NCORES)], axis=0)
    return out


# revision 7
# speedup vs baseline: 1.3387x; 1.0113x over previous
"""DynamicGraphEmbedding kernel for 8 Trainium2 NeuronCores.

The reference collapses algebraically:
  - deg[i] == K == 16 for every node (dst list is repeat(arange(N), K)),
    so gcn_norm edge weight ew == 1/16 for every edge.
  - straight-through gumbel gate is exactly y_hard in the forward pass,
    i.e. gate(e) = 1 iff argmax(softmax(logits[e] + g[e])) == 0.
  - therefore out[b] = A @ (x[b] @ W) + bias, with the dense [N, N] matrix
    A[i, j] = gate(i*N+j)/16 if j in topk_j[i] else 0.

Host (tiny, O(N^2)): build A from emb/logits/gumbel_u with the exact same
jax-on-CPU ops as the reference. Device (the memory-bound bulk): two chained
256^3 matmuls per batch element, data-parallel over batch across 8 cores.
"""

import sys

import numpy as np

if "/opt/trn_rl_repo" not in sys.path:
    sys.path.insert(0, "/opt/trn_rl_repo")

N, T, B, D, K = 256, 256, 64, 64, 16
NCORES = 8
BPC = B // NCORES  # batch elements per core

_CACHE = {}
LAST_RESULT = None  # BassKernelResults of the most recent run (for profiling)


def _graph_matrix(emb, logits, gumbel_u):
    """Dense [N, N] combined gate/topk/gcn-norm matrix A (host-side, tiny)."""
    try:
        import jax
        import jax.numpy as jnp

        cpu = jax.devices("cpu")[0]
        emb_j = jax.device_put(np.asarray(emb), cpu)
        logits_j = jax.device_put(np.asarray(logits), cpu)
        gu_j = jax.device_put(np.asarray(gumbel_u), cpu)
        nrm = jnp.linalg.norm(emb_j, axis=-1)
        cos = (emb_j @ emb_j.T) / (nrm[:, None] * nrm[None, :])
        _, topk_j = jax.lax.top_k(cos, K)
        g = -jnp.log(-jnp.log(gu_j))
        y_soft = jax.nn.softmax(logits_j + g, axis=-1)
        am = jnp.argmax(y_soft, axis=-1)
        topk = np.asarray(topk_j)
        gate_full = (np.asarray(am) == 0).astype(np.float32)
    except Exception:
        emb32 = np.asarray(emb, np.float32)
        nrm = np.sqrt((emb32 * emb32).sum(-1))
        cos = (emb32 @ emb32.T) / (nrm[:, None] * nrm[None, :])
        topk = np.argsort(-cos, axis=-1, kind="stable")[:, :K]
        lg = np.asarray(logits, np.float32) + np.float32(-1.0) * np.log(
            -np.log(np.asarray(gumbel_u, np.float32))
        )
        e = np.exp(lg - lg.max(-1, keepdims=True))
        y_soft = e / e.sum(-1, keepdims=True)
        gate_full = (np.argmax(y_soft, -1) == 0).astype(np.float32)
    rows = np.repeat(np.arange(N), K)
    cols = topk.reshape(-1)
    A = np.zeros((N, N), np.float32)
    A[rows, cols] = gate_full[rows * N + cols] * np.float32(0.0625)
    return A


def _build_bass(with_bias):
    """Per-core Bass graph: out[b] = A @ (x[b] @ W) [+ bias] for BPC batches.

    Inputs are laid out so no on-device transposes are needed:
      xT  [BPC, T, N]  x shard, transposed per batch on host
      AT  [N, N]       A transposed (contraction index on rows)
      W   [T, T]       natural
      bias [1, T]      (only when with_bias)
      out [BPC, N, T]
    """
    import concourse.bass as bass
    import concourse.mybir as mybir
    from concourse import bacc
    from concourse.tile import TileContext

    F32 = mybir.dt.float32
    # float32r: single-pass PE fp32 (TF32-ish rounding, ~1e-4 rel err) at 4x
    # the throughput of the 2-pass float32 path. PSUM accumulation stays f32.
    MMDT = mybir.dt.float32r

    nc = bacc.Bacc()
    xT = nc.declare_dram_parameter("xT", [BPC, T, N], MMDT, isOutput=False)
    AT = nc.declare_dram_parameter("AT", [N, N], MMDT, isOutput=False)
    Wp = nc.declare_dram_parameter("W", [T, T], MMDT, isOutput=False)
    if with_bias:
        bp = nc.declare_dram_parameter("bias", [1, T], F32, isOutput=False)
    out = nc.declare_dram_parameter("out", [BPC, N, T], F32, isOutput=True)

    with TileContext(nc) as tc:
        with (
            tc.tile_pool(name="const", bufs=1) as const,
            tc.tile_pool(name="xin", bufs=3) as xin,
            tc.tile_pool(name="hbuf", bufs=2) as hbuf,
            tc.tile_pool(name="obuf", bufs=4) as obuf,
            tc.tile_pool(name="psum", bufs=2, space="PSUM") as psum,
        ):
            # Constants: W and AT as [128, chunk, 256] (partition = contraction
            # index within chunk).
            w_sb = const.tile([128, 2, T], MMDT)
            nc.sync.dma_start(out=w_sb, in_=Wp.rearrange("(c p) t -> p c t", p=128))
            a_sb = const.tile([128, 2, N], MMDT)
            nc.sync.dma_start(out=a_sb, in_=AT.rearrange("(c p) n -> p c n", p=128))
            if with_bias:
                bias_bc = const.tile([128, T], F32)
                nc.gpsimd.dma_start(out=bias_bc, in_=bp.ap().to_broadcast([128, T]))

            for b in range(BPC):
                # x[b]^T as [t_in_chunk, chunk, n]
                xt = xin.tile([128, 2, N], MMDT)
                nc.sync.dma_start(
                    out=xt, in_=xT[b].rearrange("(c p) n -> p c n", p=128)
                )
                # h[b] = x[b] @ W, row blocks m of 128 nodes
                h_sb = hbuf.tile([128, 2, T], MMDT)  # [j_in_chunk, j_chunk, t']
                for m in range(2):
                    ph = psum.tile([128, T], F32)
                    nc.tensor.matmul(
                        ph,
                        lhsT=xt[:, 0, bass.ts(m, 128)],
                        rhs=w_sb[:, 0, :],
                        start=True,
                        stop=False,
                    )
                    nc.tensor.matmul(
                        ph,
                        lhsT=xt[:, 1, bass.ts(m, 128)],
                        rhs=w_sb[:, 1, :],
                        start=False,
                        stop=True,
                    )
                    nc.vector.tensor_copy(h_sb[:, m, :], ph)
                # out[b] rows block m: sum over j chunks of AT
                for m in range(2):
                    po = psum.tile([128, T], F32)
                    nc.tensor.matmul(
                        po,
                        lhsT=a_sb[:, 0, bass.ts(m, 128)],
                        rhs=h_sb[:, 0, :],
                        start=True,
                        stop=False,
                    )
                    nc.tensor.matmul(
                        po,
                        lhsT=a_sb[:, 1, bass.ts(m, 128)],
                        rhs=h_sb[:, 1, :],
                        start=False,
                        stop=True,
                    )
                    o_sb = obuf.tile([128, T], F32)
                    if with_bias:
                        nc.vector.tensor_add(o_sb, po, bias_bc)
                    else:
                        nc.vector.tensor_copy(o_sb, po)
                    nc.sync.dma_start(out=out[b, bass.ts(m, 128), :], in_=o_sb)
    nc.finalize()
    return nc


def kernel(x, emb, W, b, logits, gumbel_u):
    global LAST_RESULT
    from concourse.bass_utils import run_bass_kernel_spmd

    x = np.ascontiguousarray(np.asarray(x, np.float32))
    W = np.ascontiguousarray(np.asarray(W, np.float32))
    bias = np.ascontiguousarray(np.asarray(b, np.float32)).reshape(1, T)

    A = _graph_matrix(emb, logits, gumbel_u)
    AT = np.ascontiguousarray(A.T)
    xT = np.ascontiguousarray(x.transpose(0, 2, 1))  # [B, T, N]

    with_bias = bool(np.any(bias))
    key = ("nc", with_bias)
    if key not in _CACHE:
        _CACHE[key] = _build_bass(with_bias)
    nc = _CACHE[key]

    in_maps = [
        {"xT": xT[c * BPC : (c + 1) * BPC], "AT": AT, "W": W}
        for c in range(NCORES)
    ]
    if with_bias:
        for m in in_maps:
            m["bias"] = bias
    res = run_bass_kernel_spmd(nc, in_maps, core_ids=list(range(NCORES)))
    LAST_RESULT = res
    out = np.concatenate([res.results[c]["out"] for c in range(NCORES)], axis=0)
    return out


# revision 13
# speedup vs baseline: 1.4272x; 1.0661x over previous
"""DynamicGraphEmbedding kernel for 8 Trainium2 NeuronCores.

The reference collapses algebraically:
  - deg[i] == K == 16 for every node (dst list is repeat(arange(N), K)),
    so gcn_norm edge weight ew == 1/16 for every edge.
  - straight-through gumbel gate is exactly y_hard in the forward pass,
    i.e. gate(e) = 1 iff argmax(softmax(logits[e] + g[e])) == 0.
  - therefore out[b] = A @ (x[b] @ W) + bias, with the dense [N, N] matrix
    A[i, j] = gate(i*N+j)/16 if j in topk_j[i] else 0.

Host (tiny, O(N^2)): build A from emb/logits/gumbel_u with the exact same
jax-on-CPU ops as the reference. Device (the memory-bound bulk): two chained
256^3 matmuls per batch element, data-parallel over batch across 8 cores.
"""

import sys

import numpy as np

if "/opt/trn_rl_repo" not in sys.path:
    sys.path.insert(0, "/opt/trn_rl_repo")

N, T, B, D, K = 256, 256, 64, 64, 16
NCORES = 8
BPC = B // NCORES  # batch elements per core

_CACHE = {}
LAST_RESULT = None  # BassKernelResults of the most recent run (for profiling)


def _graph_matrix(emb, logits, gumbel_u):
    """Dense [N, N] combined gate/topk/gcn-norm matrix A (host-side, tiny)."""
    try:
        import jax
        import jax.numpy as jnp

        cpu = jax.devices("cpu")[0]
        emb_j = jax.device_put(np.asarray(emb), cpu)
        logits_j = jax.device_put(np.asarray(logits), cpu)
        gu_j = jax.device_put(np.asarray(gumbel_u), cpu)
        nrm = jnp.linalg.norm(emb_j, axis=-1)
        cos = (emb_j @ emb_j.T) / (nrm[:, None] * nrm[None, :])
        _, topk_j = jax.lax.top_k(cos, K)
        g = -jnp.log(-jnp.log(gu_j))
        y_soft = jax.nn.softmax(logits_j + g, axis=-1)
        am = jnp.argmax(y_soft, axis=-1)
        topk = np.asarray(topk_j)
        gate_full = (np.asarray(am) == 0).astype(np.float32)
    except Exception:
        emb32 = np.asarray(emb, np.float32)
        nrm = np.sqrt((emb32 * emb32).sum(-1))
        cos = (emb32 @ emb32.T) / (nrm[:, None] * nrm[None, :])
        topk = np.argsort(-cos, axis=-1, kind="stable")[:, :K]
        lg = np.asarray(logits, np.float32) + np.float32(-1.0) * np.log(
            -np.log(np.asarray(gumbel_u, np.float32))
        )
        e = np.exp(lg - lg.max(-1, keepdims=True))
        y_soft = e / e.sum(-1, keepdims=True)
        gate_full = (np.argmax(y_soft, -1) == 0).astype(np.float32)
    rows = np.repeat(np.arange(N), K)
    cols = topk.reshape(-1)
    A = np.zeros((N, N), np.float32)
    A[rows, cols] = gate_full[rows * N + cols] * np.float32(0.0625)
    return A


NG = BPC // 2  # batch pairs per core


def _build_bass(with_bias):
    """Per-core Bass graph: out[b] = A @ (x[b] @ W) [+ bias] for BPC batches.

    Host-packed layouts (8KB contiguous per-partition runs, few big DMAs):
      consts [128, 4, 256]        [p, g, t]: g = (W c0, W c1, AT c0, AT c1)
      xin    [NG, 2, 128, 2, 256] [g, c, p, bi, n] = x[2g+bi][n, c*128+p]
      bias   [1, 256]             (only when with_bias)
      out    [BPC, N, T]          natural layout, written straight from PSUM
    """
    import concourse.bass as bass
    import concourse.mybir as mybir
    from concourse import bacc
    from concourse.tile import TileContext

    F32 = mybir.dt.float32
    # float32r: single-pass PE fp32 (TF32-ish rounding, ~1e-4 rel err) at 4x
    # the throughput of the 2-pass float32 path. PSUM accumulation stays f32.
    MMDT = mybir.dt.float32r

    nc = bacc.Bacc()
    consts = nc.declare_dram_parameter("consts", [128, 4, 256], MMDT, isOutput=False)
    xin = nc.declare_dram_parameter("xin", [NG, 2, 128, 2, N], MMDT, isOutput=False)
    if with_bias:
        bp = nc.declare_dram_parameter("bias", [1, T], F32, isOutput=False)
    out = nc.declare_dram_parameter("out", [BPC, N, T], F32, isOutput=True)

    with TileContext(nc) as tc:
        with (
            tc.tile_pool(name="const", bufs=1) as const,
            tc.tile_pool(name="xpool", bufs=2) as xpool,
            tc.tile_pool(name="hbuf", bufs=2) as hbuf,
            tc.tile_pool(name="obuf", bufs=3) as obuf,
            tc.tile_pool(name="psA", bufs=4, space="PSUM") as psA,
            tc.tile_pool(name="psB", bufs=2, space="PSUM") as psB,
        ):
            ct = const.tile([128, 4, 256], MMDT)
            nc.sync.dma_start(out=ct, in_=consts[:, :, :])
            if with_bias:
                bias_bc = const.tile([128, T], F32)
                nc.gpsimd.dma_start(out=bias_bc, in_=bp.ap().to_broadcast([128, T]))

            for g in range(NG):
                xt = xpool.tile([128, 2, 2, N], MMDT)  # [p=t%128, c, bi, n]
                nc.sync.dma_start(
                    out=xt, in_=xin[g].rearrange("c p bi n -> p c bi n")
                )
                # h for the pair: [p=j%128, jc(=node block m), bi, t']
                h_sb = hbuf.tile([128, 2, 2, T], MMDT)
                for bi in range(2):
                    for m in range(2):
                        ph = psA.tile([128, T], F32)
                        nc.tensor.matmul(
                            ph,
                            lhsT=xt[:, 0, bi, bass.ts(m, 128)],
                            rhs=ct[:, 0, :],
                            start=True,
                            stop=False,
                        )
                        nc.tensor.matmul(
                            ph,
                            lhsT=xt[:, 1, bi, bass.ts(m, 128)],
                            rhs=ct[:, 1, :],
                            start=False,
                            stop=True,
                        )
                        nc.vector.tensor_copy(h_sb[:, m, bi, :], ph)
                for m in range(2):
                    po = psB.tile([128, 2, T], F32)  # [n%128, bi, t'] one bank
                    nc.tensor.matmul(
                        po,
                        lhsT=ct[:, 2, bass.ts(m, 128)],
                        rhs=h_sb[:, 0, :, :],
                        start=True,
                        stop=False,
                    )
                    nc.tensor.matmul(
                        po,
                        lhsT=ct[:, 3, bass.ts(m, 128)],
                        rhs=h_sb[:, 1, :, :],
                        start=False,
                        stop=True,
                    )
                    ob = obuf.tile([128, 2, T], F32)
                    if with_bias:
                        for bi in range(2):
                            nc.vector.tensor_add(ob[:, bi, :], po[:, bi, :], bias_bc)
                    else:
                        nc.vector.tensor_copy(ob, po)
                    # store on the ACT hwdge queue (SP handles the loads)
                    nc.scalar.dma_start(
                        out=out[2 * g : 2 * g + 2, bass.ts(m, 128), :].rearrange(
                            "b p t -> p b t"
                        ),
                        in_=ob,
                    )
    nc.finalize()
    return nc


def kernel(x, emb, W, b, logits, gumbel_u):
    global LAST_RESULT
    from concourse.bass_utils import run_bass_kernel_spmd

    x = np.asarray(x, np.float32)
    W = np.asarray(W, np.float32)
    bias = np.ascontiguousarray(np.asarray(b, np.float32)).reshape(1, T)

    A = _graph_matrix(emb, logits, gumbel_u)
    # consts [128, 4, 256]: W chunks then AT chunks, partition-major
    Wr = W.reshape(2, 128, T).transpose(1, 0, 2)
    Ar = np.ascontiguousarray(A.T).reshape(2, 128, N).transpose(1, 0, 2)
    consts = np.ascontiguousarray(np.concatenate([Wr, Ar], axis=1))

    # xin [B/2 pairs, c, p, bi, n]: xT[b][t, n] split t = c*128+p, b = 2g+bi
    xT = x.transpose(0, 2, 1)  # [B, T, N]
    xpack = np.ascontiguousarray(
        xT.reshape(B // 2, 2, 2, 128, N).transpose(0, 2, 3, 1, 4)
    )

    with_bias = bool(np.any(bias))
    key = ("nc", with_bias)
    if key not in _CACHE:
        _CACHE[key] = _build_bass(with_bias)
    nc = _CACHE[key]

    in_maps = [
        {"xin": xpack[c * NG : (c + 1) * NG], "consts": consts}
        for c in range(NCORES)
    ]
    if with_bias:
        for m in in_maps:
            m["bias"] = bias
    res = run_bass_kernel_spmd(nc, in_maps, core_ids=list(range(NCORES)))
    LAST_RESULT = res
    out = np.concatenate([res.results[c]["out"] for c in range(NCORES)], axis=0)
    return out
